# revision 28
# baseline (speedup 1.0000x reference)
"""AdaptiveDensityTokenizer on 8 TRN2 NeuronCores.

Strategy: the memory-bound importance MLP (reads all of `features`,
64 MB) is K-sharded across the 8 cores (4096 points/core); each core
computes softplus(relu(feat^T @ W1 + b1) @ W2 + b2) for its points.
The tiny data-dependent sequential logic (spatial bucketing, per-region
farthest-point sampling of ~256 total picks, concat/scatter) runs on
host, exactly replicating the reference semantics. The final token
projection (256 tokens x 256 feat @ Wa) is also host-side (0.03% of
the FLOPs).
"""

import os
import sys

import numpy as np

for _p in ("/opt/trn_rl_repo", "/root/.axon_site/_ro/trn_rl_repo"):
    if os.path.isdir(_p) and _p not in sys.path:
        sys.path.append(_p)

NCORES = 8
B, K, C, D = 2, 32768, 256, 256
T_TOK = 256
RPD = 3
R3 = 27
KC = K // NCORES          # points per core
CH = C // 128             # contraction chunks of 128
H = C // 2                # hidden width = 128
PT = 512                  # points per matmul tile (one PSUM bank of f32)

_NC = None                # cached compiled Bass program
LAST = None               # last BassKernelResults (for profiling)


def _build_mlp():
    """One SPMD program: per-core importance MLP over a K-shard.

    Layout/perf design (measured on neuron-profile traces):
    - features arrive host-packed as (128, 4*KC) bf16 so every DMA
      descriptor is a 4 KB contiguous row; staging is split into 8
      chunk DMAs alternating between the SP and ACT HWDGE rings (two
      descriptor generators) to feed all 16 SDMA engines.
    - a packed single-DMA weight load precedes the chunk triggers.
    - dependency-free scratch matmuls pre-heat the PE during the
      staging window so the HAM clock gate (1.2 -> 2.4 GHz after ~4us
      of sustained activity) lifts before real matmuls start.
    - mm1 accumulates C=256 in two 128-chunks into (128,1024) PSUM
      tiles; bias+relu drains alternate between ACT (fused activation)
      and DVE (fused tensor_scalar add+max); hs is written bf16.
    - the 16 (1,512) mm2 results land on PSUM partitions {0,32,64} of
      grouped tiles (PE writes must base at 0/32/64), drained eagerly
      by wide DVE copies + partition-strided DMAs; groups are sized
      {3,3,1,1} so the final drain chain after the last matmul is
      minimal.
    """
    from concourse import bacc, mybir, tile

    f32 = mybir.dt.float32
    bf16 = mybir.dt.bfloat16
    nc = bacc.Bacc(None, target_bir_lowering=False, debug=False)

    feat = nc.declare_dram_parameter("features", [128, 4 * KC], bf16, False)
    w1 = nc.declare_dram_parameter("W1", [128, C + 1], bf16, False)
    b1 = nc.declare_dram_parameter("b1", [H, 1], f32, False)
    out = nc.declare_dram_parameter("z", [B, KC], f32, True)

    # packed quarters (must match _pack_features): quarter q holds
    # BOTH c-chunks for 2048 points: [c0 | c1] at col q*4096. One
    # full-partition 1 MB DMA per quarter (8 KB descriptors), quarters
    # alternating rings, so each ring delivers complete b-halves and
    # the two halves of b0 arrive simultaneously on parallel rings.
    CHUNKS = [("A", 0), ("B", 4096), ("C", 8192), ("D", 12288)]

    with tile.TileContext(nc) as tc:
        with (
            tc.tile_pool(name="wpool", bufs=1) as wpool,
            tc.tile_pool(name="fpool", bufs=1) as fpool,
            tc.tile_pool(name="hpool", bufs=3) as hpool,
            tc.tile_pool(name="zpool", bufs=2) as zpool,
            tc.tile_pool(name="ph", bufs=2, space="PSUM") as phpool,
            tc.tile_pool(name="pz", bufs=1, space="PSUM") as pzpool,
        ):
            # Weights first (single packed DMA on the sync ring; they gate
            # the first real matmuls), then the staging chunk triggers so
            # both HWDGE rings stream features immediately after.
            wall = wpool.tile([128, C + 1], bf16, tag="wall")
            nc.sync.dma_start(out=wall[:], in_=w1[:, :])
            b1t = wpool.tile([H, 1], f32, tag="b1")
            nc.scalar.dma_start(out=b1t[:], in_=b1[:, :])
            w1t = [wall[:, c * 128:(c + 1) * 128] for c in range(CH)]
            w2t = wall[:, C:C + 1]

            ftiles = {}
            for i, (name, off) in enumerate(CHUNKS):
                t = fpool.tile([128, 4096], bf16, tag=f"f_{name}")
                eng = nc.sync if i % 2 == 0 else nc.scalar
                eng.dma_start(out=t[:], in_=feat[:, off:off + 4096])
                ftiles[name] = t

            def rhs(b, c, j):
                # feature columns [j, j+512) of chunk c for batch b
                t = ftiles[("AB", "CD")[b][j // 2048]]
                col = c * 2048 + (j % 2048)
                return t[:, col:col + 512]

            # Pre-heat the PE during the staging window: the HAM clock
            # gate needs ~4us of sustained activity to lift 1.2->2.4 GHz,
            # and the first real matmul can't start until weights+chunks
            # land (~10us). Scratch matmuls on uninitialized tiles have no
            # dependencies and keep the array busy from right after the
            # start barrier, so real work runs at the warm clock.
            wscr = wpool.tile([128, 128], bf16, tag="wscr")
            mscr = wpool.tile([128, PT], bf16, tag="mscr")
            nc.vector.memset(wscr[:], 0.0)
            nc.vector.memset(mscr[:], 0.0)
            pwarm = pzpool.tile([65, PT], f32, tag="pz0", name="pwarm")
            for _ in range(14):
                nc.tensor.matmul(pwarm[0:1, :], wscr[:, 0:1], mscr[:],
                                 start=True, stop=True)

            relu = mybir.ActivationFunctionType.Relu
            add = mybir.AluOpType.add
            amax = mybir.AluOpType.max
            for b in range(B):
                # mm2 outputs must land on PSUM base partition 0/32/64:
                # pack 3 per PSUM tile, drain with partition-strided copies.
                pzs = []
                for t in range(4):
                    pzt = pzpool.tile([65, PT], f32, tag=f"pz{t}", name=f"pz{t}_{b}")
                    pzs.append(pzt)
                for i in range(4):                      # 1024-pt double tiles
                    ph = phpool.tile([128, 2 * PT], f32)
                    hs = hpool.tile([128, 2 * PT], bf16)
                    for half in range(2):
                        sl = slice(half * PT, (half + 1) * PT)
                        j = i * 1024 + half * PT
                        for c in range(CH):
                            nc.tensor.matmul(
                                ph[:, sl], w1t[c], rhs(b, c, j),
                                start=(c == 0), stop=(c == CH - 1),
                            )
                        if half == 0:
                            nc.scalar.activation(hs[:, sl], ph[:, sl], relu,
                                                 bias=b1t[:])
                        else:
                            nc.vector.tensor_scalar(
                                out=hs[:, sl], in0=ph[:, sl],
                                scalar1=b1t[:], scalar2=0.0,
                                op0=add, op1=amax,
                            )
                        r = 2 * i + half
                        grp = (0, 0, 0, 1, 1, 1, 2, 3)[r]
                        base = (0, 3, 6, 7)[grp]
                        row = (r - base) * 32
                        nc.tensor.matmul(
                            pzs[grp][row:row + 1, :], w2t, hs[:, sl],
                            start=True, stop=True,
                        )
                        if r in (2, 5, 6, 7):   # group full: drain eagerly
                            grp = (0, 0, 0, 1, 1, 1, 2, 3)[r]
                            nrow = (3, 3, 1, 1)[grp]
                            zview = out[b].rearrange("(p f) -> p f", p=8)
                            zt = zpool.tile([65, PT], f32, tag=f"zsb{grp}",
                                            name=f"zsb{grp}_{b}")
                            nc.vector.tensor_copy(
                                zt[0:(nrow - 1) * 32 + 1, :],
                                pzs[grp][0:(nrow - 1) * 32 + 1, :])
                            nc.sync.dma_start(
                                out=zview[base:base + nrow, :],
                                in_=zt[0:(nrow - 1) * 32 + 1:32, :],
                            )

    nc.compile()
    return nc


def _pack_features(fbf, core):
    """(B,C,K) bf16 -> per-core (128, 4*KC) packed layout; see CHUNKS."""
    sh = fbf[:, :, core * KC:(core + 1) * KC].reshape(B, CH, 128, KC)
    p = np.empty((128, 4 * KC), fbf.dtype)
    for q, (b, j0) in enumerate(((0, 0), (0, 2048), (1, 0), (1, 2048))):
        p[:, q * 4096:q * 4096 + 2048] = sh[b, 0, :, j0:j0 + 2048]
        p[:, q * 4096 + 2048:q * 4096 + 4096] = sh[b, 1, :, j0:j0 + 2048]
    return p


def _get_nc():
    global _NC
    if _NC is None:
        _NC = _build_mlp()
    return _NC


def _ensure_profile_hook():
    """Shim antenv.axon_hooks (absent in this image) so the trace=True
    path of run_bass_kernel_spmd can capture NTFF profiles, and stub the
    S3 artifact upload. Only used when BASS_PROFILE=1."""
    import types

    try:
        import antenv.axon_hooks  # noqa: F401
    except ImportError:
        try:
            import antenv
            from trn_agent_boot.trn_boot import _ntff_profile_via_ctypes

            hook = _ntff_profile_via_ctypes("/opt/axon/libaxon_pjrt.so")
            mod = types.ModuleType("antenv.axon_hooks")
            mod.get_axon_ntff_profile_hook = lambda: hook
            mod.set_axon_ntff_profile_hook = lambda h: None
            sys.modules["antenv.axon_hooks"] = mod
            antenv.axon_hooks = mod
        except Exception:
            return False
    import concourse.bass_utils as bu

    bu.upload_artifacts = lambda d: "file://" + d
    return True


def _device_importance(features, W1, b1, W2, b2):
    from concourse.bass_utils import run_bass_kernel_spmd

    global LAST
    nc = _get_nc()
    profile = bool(int(os.environ.get("BASS_PROFILE", "0")))
    if profile:
        profile = _ensure_profile_hook()
    import ml_dtypes

    bf = ml_dtypes.bfloat16
    w1p = np.empty((128, C + 1), bf)
    w1p[:, 0:H] = np.ascontiguousarray(W1[0:128]).astype(bf)
    w1p[:, H:C] = np.ascontiguousarray(W1[128:256]).astype(bf)
    w1p[:, C] = np.ascontiguousarray(W2).astype(bf).reshape(H)
    b1c = np.ascontiguousarray(b1, np.float32).reshape(H, 1)
    fbf = np.asarray(features, np.float32).astype(bf)
    in_maps = []
    for core in range(NCORES):
        sl = slice(core * KC, (core + 1) * KC)
        in_maps.append({
            "features": _pack_features(fbf, core),
            "W1": w1p, "b1": b1c,
        })
    res = run_bass_kernel_spmd(
        nc, in_maps, core_ids=list(range(NCORES)), trace=profile,
    )
    LAST = res
    z = np.concatenate([res.results[c]["z"] for c in range(NCORES)], axis=1)
    z = z + np.float32(np.asarray(b2).reshape(()))
    # softplus on host with the exact jax.nn.softplus formula in f32
    # (the ACT engine's Softplus LUT is not precise enough for the
    # data-dependent n_r rounding margins).
    return np.maximum(z, 0) + np.log1p(np.exp(-np.abs(z)))


def _fps_region(pts, n_steps):
    """Farthest-point sampling over one compacted region, mirroring the
    reference: start at subset index 0, squared L2, first-max argmax."""
    n = pts.shape[0]
    picks = np.empty(n_steps, np.int64)
    mind = np.full(n, np.float32(1e10), np.float32)
    p = 0
    x, y, z = pts[:, 0], pts[:, 1], pts[:, 2]
    for s in range(n_steps):
        picks[s] = p
        dx = x - x[p]
        dy = y - y[p]
        dz = z - z[p]
        d = (dx * dx + dy * dy) + dz * dz
        np.minimum(mind, d, out=mind)
        p = int(np.argmax(mind))
    return picks


def kernel(xyz, features, W1, b1, W2, b2, Wa, ba):
    xyz = np.asarray(xyz, np.float32)
    features = np.asarray(features, np.float32)
    Wa = np.asarray(Wa, np.float32)
    ba = np.asarray(ba, np.float32)

    imp = _device_importance(features, W1, b1, W2, b2)        # (B, K)

    # ---- spatial bucketing (exact reference semantics, f32 ops) ----
    mn = xyz.min(axis=1, keepdims=True)
    mx = xyz.max(axis=1, keepdims=True)
    xn = (xyz - mn) / (mx - mn + np.float32(1e-6))
    ridx = np.clip(xn * np.float32(RPD), 0, RPD - 1).astype(np.int32)
    rid = ridx[..., 0] * RPD * RPD + ridx[..., 1] * RPD + ridx[..., 2]
    valid = np.abs(xyz).sum(-1) > 0                           # (B, K)

    onehot = (rid[..., None] == np.arange(R3)) & valid[..., None]
    counts = onehot.sum(axis=1).astype(np.int32)              # (B, R3)
    reg_imp = np.einsum(
        "bk,bkr->br",
        (imp * valid).astype(np.float32),
        onehot.astype(np.float32),
    )
    share = reg_imp / (reg_imp.sum(-1, keepdims=True) + np.float32(1e-8))
    n_r = np.round(share * np.float32(T_TOK)).astype(np.int32)
    c_r = np.where(n_r == 0, 0, np.minimum(n_r, counts))      # (B, R3)

    # ---- per-region selection: FPS picks or ascending slab ----
    out_idx = np.zeros((B, T_TOK), np.int32)
    filled = np.zeros((B, T_TOK), bool)
    for b in range(B):
        start = 0
        for r in range(R3):
            c = int(c_r[b, r])
            if c == 0:
                continue
            members = np.nonzero(onehot[b, :, r])[0]          # ascending
            if counts[b, r] <= n_r[b, r]:
                sel = members[:c]
            else:
                pts = xyz[b][members]
                sel = members[_fps_region(pts, c)]
            take = min(c, T_TOK - start)
            if take > 0:
                out_idx[b, start:start + take] = sel[:take]
                filled[b, start:start + take] = True
            start += c
            if start >= T_TOK:
                break

    # ---- gather + output heads ----
    xyz_tok = np.where(
        filled[..., None], np.take_along_axis(xyz, out_idx[..., None], axis=1), 0.0
    ).astype(np.float32)

    gath = np.stack([features[b][:, out_idx[b]].T for b in range(B)])  # (B,T,C)
    tok = gath @ Wa + ba                                               # (B,T,D)
    feat_tok = np.where(filled[..., None], tok, 0.0).transpose(0, 2, 1)
    return xyz_tok.astype(np.float32), feat_tok.astype(np.float32)


# revision 29
# speedup vs baseline: 1.1248x; 1.1248x over previous
"""AdaptiveDensityTokenizer on 8 TRN2 NeuronCores.

Strategy: the memory-bound importance MLP (reads all of `features`,
64 MB) is K-sharded across the 8 cores (4096 points/core); each core
computes softplus(relu(feat^T @ W1 + b1) @ W2 + b2) for its points.
The tiny data-dependent sequential logic (spatial bucketing, per-region
farthest-point sampling of ~256 total picks, concat/scatter) runs on
host, exactly replicating the reference semantics. The final token
projection (256 tokens x 256 feat @ Wa) is also host-side (0.03% of
the FLOPs).
"""

import os
import sys

import numpy as np

for _p in ("/opt/trn_rl_repo", "/root/.axon_site/_ro/trn_rl_repo"):
    if os.path.isdir(_p) and _p not in sys.path:
        sys.path.append(_p)

NCORES = 8
B, K, C, D = 2, 32768, 256, 256
T_TOK = 256
RPD = 3
R3 = 27
KC = K // NCORES          # points per core
CH = C // 128             # contraction chunks of 128
H = C // 2                # hidden width = 128
PT = 512                  # points per matmul tile (one PSUM bank of f32)

_NC = None                # cached compiled Bass program
LAST = None               # last BassKernelResults (for profiling)


def _build_mlp():
    """One SPMD program: per-core importance MLP over a K-shard.

    Layout/perf design (measured on neuron-profile traces):
    - features arrive host-packed as (128, 4*KC) bf16 so every DMA
      descriptor is a 4 KB contiguous row; staging is split into 8
      chunk DMAs alternating between the SP and ACT HWDGE rings (two
      descriptor generators) to feed all 16 SDMA engines.
    - a packed single-DMA weight load precedes the chunk triggers.
    - dependency-free scratch matmuls pre-heat the PE during the
      staging window so the HAM clock gate (1.2 -> 2.4 GHz after ~4us
      of sustained activity) lifts before real matmuls start.
    - mm1 accumulates C=256 in two 128-chunks into (128,1024) PSUM
      tiles; bias+relu drains alternate between ACT (fused activation)
      and DVE (fused tensor_scalar add+max); hs is written bf16.
    - the 16 (1,512) mm2 results land on PSUM partitions {0,32,64} of
      grouped tiles (PE writes must base at 0/32/64), drained eagerly
      by wide DVE copies + partition-strided DMAs; groups are sized
      {3,3,1,1} so the final drain chain after the last matmul is
      minimal.
    """
    from concourse import bacc, mybir, tile

    f32 = mybir.dt.float32
    bf16 = mybir.dt.bfloat16
    nc = bacc.Bacc(None, target_bir_lowering=False, debug=False)

    feat = nc.declare_dram_parameter("features", [128, 4 * KC], bf16, False)
    w1 = nc.declare_dram_parameter("W1", [128, C + 1], bf16, False)
    b1 = nc.declare_dram_parameter("b1", [H, 1], f32, False)
    out = nc.declare_dram_parameter("z", [B, KC], f32, True)

    # packed column ranges (must match _pack_features):
    #   [b0c0a | b0c1a | b0c0b | b0c1b | b1c0a | b1c1a | b1c0b | b1c1b]
    # 2048-wide chunks (4 KB descriptor rows) measured faster than any
    # 4096-wide variant: descriptor generation scales with size, and
    # finer granularity keeps the PE fed earlier.
    CHUNKS = [
        ("A0", 0, 2048), ("A1", 2048, 2048),
        ("B0", 4096, 2048), ("B1", 6144, 2048),
        ("C0", 8192, 2048), ("C1", 10240, 2048),
        ("D0", 12288, 2048), ("D1", 14336, 2048),
    ]

    with tile.TileContext(nc) as tc:
        with (
            tc.tile_pool(name="wpool", bufs=1) as wpool,
            tc.tile_pool(name="fpool", bufs=1) as fpool,
            tc.tile_pool(name="hpool", bufs=3) as hpool,
            tc.tile_pool(name="zpool", bufs=2) as zpool,
            tc.tile_pool(name="ph", bufs=2, space="PSUM") as phpool,
            tc.tile_pool(name="pz", bufs=1, space="PSUM") as pzpool,
        ):
            # Weights first (single packed DMA on the sync ring; they gate
            # the first real matmuls), then the staging chunk triggers so
            # both HWDGE rings stream features immediately after.
            wall = wpool.tile([128, C + 1], bf16, tag="wall")
            nc.sync.dma_start(out=wall[:], in_=w1[:, :])
            b1t = wpool.tile([H, 1], f32, tag="b1")
            nc.scalar.dma_start(out=b1t[:], in_=b1[:, :])
            w1t = [wall[:, c * 128:(c + 1) * 128] for c in range(CH)]
            w2t = wall[:, C:C + 1]

            ftiles = {}
            for i, (name, off, width) in enumerate(CHUNKS):
                t = fpool.tile([128, width], bf16, tag=f"f_{name}")
                eng = nc.sync if i % 2 == 0 else nc.scalar
                eng.dma_start(out=t[:], in_=feat[:, off:off + width])
                ftiles[name] = t

            def rhs(b, c, j):
                # feature columns [j, j+512) of chunk c for batch b
                names = (("A0", "A1", "B0", "B1"), ("C0", "C1", "D0", "D1"))
                t = ftiles[names[b][(j // 2048) * 2 + c]]
                return t[:, j % 2048:j % 2048 + 512]

            # Pre-heat the PE during the staging window: the HAM clock
            # gate needs ~4us of sustained activity to lift 1.2->2.4 GHz,
            # and the first real matmul can't start until weights+chunks
            # land (~10us). Scratch matmuls on uninitialized tiles have no
            # dependencies and keep the array busy from right after the
            # start barrier, so real work runs at the warm clock.
            wscr = wpool.tile([128, 128], bf16, tag="wscr")
            mscr = wpool.tile([128, PT], bf16, tag="mscr")
            nc.vector.memset(wscr[:], 0.0)
            nc.vector.memset(mscr[:], 0.0)
            pwarm = pzpool.tile([65, PT], f32, tag="pz0", name="pwarm")
            for _ in range(14):
                nc.tensor.matmul(pwarm[0:1, :], wscr[:, 0:1], mscr[:],
                                 start=True, stop=True)

            relu = mybir.ActivationFunctionType.Relu
            add = mybir.AluOpType.add
            amax = mybir.AluOpType.max
            for b in range(B):
                # mm2 outputs must land on PSUM base partition 0/32/64:
                # pack 3 per PSUM tile, drain with partition-strided copies.
                pzs = []
                for t in range(4):
                    pzt = pzpool.tile([65, PT], f32, tag=f"pz{t}", name=f"pz{t}_{b}")
                    pzs.append(pzt)
                for i in range(4):                      # 1024-pt double tiles
                    ph = phpool.tile([128, 2 * PT], f32)
                    hs = hpool.tile([128, 2 * PT], bf16)
                    for half in range(2):
                        sl = slice(half * PT, (half + 1) * PT)
                        j = i * 1024 + half * PT
                        for c in range(CH):
                            nc.tensor.matmul(
                                ph[:, sl], w1t[c], rhs(b, c, j),
                                start=(c == 0), stop=(c == CH - 1),
                            )
                        if half == 0:
                            nc.scalar.activation(hs[:, sl], ph[:, sl], relu,
                                                 bias=b1t[:])
                        else:
                            nc.vector.tensor_scalar(
                                out=hs[:, sl], in0=ph[:, sl],
                                scalar1=b1t[:], scalar2=0.0,
                                op0=add, op1=amax,
                            )
                        r = 2 * i + half
                        grp = (0, 0, 0, 1, 1, 1, 2, 3)[r]
                        base = (0, 3, 6, 7)[grp]
                        row = (r - base) * 32
                        nc.tensor.matmul(
                            pzs[grp][row:row + 1, :], w2t, hs[:, sl],
                            start=True, stop=True,
                        )
                        if r in (2, 5, 6, 7):   # group full: drain eagerly
                            grp = (0, 0, 0, 1, 1, 1, 2, 3)[r]
                            nrow = (3, 3, 1, 1)[grp]
                            zview = out[b].rearrange("(p f) -> p f", p=8)
                            zt = zpool.tile([65, PT], f32, tag=f"zsb{grp}",
                                            name=f"zsb{grp}_{b}")
                            nc.vector.tensor_copy(
                                zt[0:(nrow - 1) * 32 + 1, :],
                                pzs[grp][0:(nrow - 1) * 32 + 1, :])
                            nc.sync.dma_start(
                                out=zview[base:base + nrow, :],
                                in_=zt[0:(nrow - 1) * 32 + 1:32, :],
                            )

    nc.compile()
    return nc


def _pack_features(fbf, core):
    """(B,C,K) bf16 -> per-core (128, 4*KC) packed layout; see CHUNKS."""
    sh = fbf[:, :, core * KC:(core + 1) * KC].reshape(B, CH, 128, KC)
    p = np.empty((128, 4 * KC), fbf.dtype)
    p[:, 0:2048] = sh[0, 0, :, 0:2048]
    p[:, 2048:4096] = sh[0, 1, :, 0:2048]
    p[:, 4096:6144] = sh[0, 0, :, 2048:4096]
    p[:, 6144:8192] = sh[0, 1, :, 2048:4096]
    p[:, 8192:10240] = sh[1, 0, :, 0:2048]
    p[:, 10240:12288] = sh[1, 1, :, 0:2048]
    p[:, 12288:14336] = sh[1, 0, :, 2048:4096]
    p[:, 14336:16384] = sh[1, 1, :, 2048:4096]
    return p


def _get_nc():
    global _NC
    if _NC is None:
        _NC = _build_mlp()
    return _NC


def _ensure_profile_hook():
    """Shim antenv.axon_hooks (absent in this image) so the trace=True
    path of run_bass_kernel_spmd can capture NTFF profiles, and stub the
    S3 artifact upload. Only used when BASS_PROFILE=1."""
    import types

    try:
        import antenv.axon_hooks  # noqa: F401
    except ImportError:
        try:
            import antenv
            from trn_agent_boot.trn_boot import _ntff_profile_via_ctypes

            hook = _ntff_profile_via_ctypes("/opt/axon/libaxon_pjrt.so")
            mod = types.ModuleType("antenv.axon_hooks")
            mod.get_axon_ntff_profile_hook = lambda: hook
            mod.set_axon_ntff_profile_hook = lambda h: None
            sys.modules["antenv.axon_hooks"] = mod
            antenv.axon_hooks = mod
        except Exception:
            return False
    import concourse.bass_utils as bu

    bu.upload_artifacts = lambda d: "file://" + d
    return True


def _device_importance(features, W1, b1, W2, b2):
    from concourse.bass_utils import run_bass_kernel_spmd

    global LAST
    nc = _get_nc()
    profile = bool(int(os.environ.get("BASS_PROFILE", "0")))
    if profile:
        profile = _ensure_profile_hook()
    import ml_dtypes

    bf = ml_dtypes.bfloat16
    w1p = np.empty((128, C + 1), bf)
    w1p[:, 0:H] = np.ascontiguousarray(W1[0:128]).astype(bf)
    w1p[:, H:C] = np.ascontiguousarray(W1[128:256]).astype(bf)
    w1p[:, C] = np.ascontiguousarray(W2).astype(bf).reshape(H)
    b1c = np.ascontiguousarray(b1, np.float32).reshape(H, 1)
    fbf = np.asarray(features, np.float32).astype(bf)
    in_maps = []
    for core in range(NCORES):
        sl = slice(core * KC, (core + 1) * KC)
        in_maps.append({
            "features": _pack_features(fbf, core),
            "W1": w1p, "b1": b1c,
        })
    res = run_bass_kernel_spmd(
        nc, in_maps, core_ids=list(range(NCORES)), trace=profile,
    )
    LAST = res
    z = np.concatenate([res.results[c]["z"] for c in range(NCORES)], axis=1)
    z = z + np.float32(np.asarray(b2).reshape(()))
    # softplus on host with the exact jax.nn.softplus formula in f32
    # (the ACT engine's Softplus LUT is not precise enough for the
    # data-dependent n_r rounding margins).
    return np.maximum(z, 0) + np.log1p(np.exp(-np.abs(z)))


def _fps_region(pts, n_steps):
    """Farthest-point sampling over one compacted region, mirroring the
    reference: start at subset index 0, squared L2, first-max argmax."""
    n = pts.shape[0]
    picks = np.empty(n_steps, np.int64)
    mind = np.full(n, np.float32(1e10), np.float32)
    p = 0
    x, y, z = pts[:, 0], pts[:, 1], pts[:, 2]
    for s in range(n_steps):
        picks[s] = p
        dx = x - x[p]
        dy = y - y[p]
        dz = z - z[p]
        d = (dx * dx + dy * dy) + dz * dz
        np.minimum(mind, d, out=mind)
        p = int(np.argmax(mind))
    return picks


def kernel(xyz, features, W1, b1, W2, b2, Wa, ba):
    xyz = np.asarray(xyz, np.float32)
    features = np.asarray(features, np.float32)
    Wa = np.asarray(Wa, np.float32)
    ba = np.asarray(ba, np.float32)

    imp = _device_importance(features, W1, b1, W2, b2)        # (B, K)

    # ---- spatial bucketing (exact reference semantics, f32 ops) ----
    mn = xyz.min(axis=1, keepdims=True)
    mx = xyz.max(axis=1, keepdims=True)
    xn = (xyz - mn) / (mx - mn + np.float32(1e-6))
    ridx = np.clip(xn * np.float32(RPD), 0, RPD - 1).astype(np.int32)
    rid = ridx[..., 0] * RPD * RPD + ridx[..., 1] * RPD + ridx[..., 2]
    valid = np.abs(xyz).sum(-1) > 0                           # (B, K)

    onehot = (rid[..., None] == np.arange(R3)) & valid[..., None]
    counts = onehot.sum(axis=1).astype(np.int32)              # (B, R3)
    reg_imp = np.einsum(
        "bk,bkr->br",
        (imp * valid).astype(np.float32),
        onehot.astype(np.float32),
    )
    share = reg_imp / (reg_imp.sum(-1, keepdims=True) + np.float32(1e-8))
    n_r = np.round(share * np.float32(T_TOK)).astype(np.int32)
    c_r = np.where(n_r == 0, 0, np.minimum(n_r, counts))      # (B, R3)

    # ---- per-region selection: FPS picks or ascending slab ----
    out_idx = np.zeros((B, T_TOK), np.int32)
    filled = np.zeros((B, T_TOK), bool)
    for b in range(B):
        start = 0
        for r in range(R3):
            c = int(c_r[b, r])
            if c == 0:
                continue
            members = np.nonzero(onehot[b, :, r])[0]          # ascending
            if counts[b, r] <= n_r[b, r]:
                sel = members[:c]
            else:
                pts = xyz[b][members]
                sel = members[_fps_region(pts, c)]
            take = min(c, T_TOK - start)
            if take > 0:
                out_idx[b, start:start + take] = sel[:take]
                filled[b, start:start + take] = True
            start += c
            if start >= T_TOK:
                break

    # ---- gather + output heads ----
    xyz_tok = np.where(
        filled[..., None], np.take_along_axis(xyz, out_idx[..., None], axis=1), 0.0
    ).astype(np.float32)

    gath = np.stack([features[b][:, out_idx[b]].T for b in range(B)])  # (B,T,C)
    tok = gath @ Wa + ba                                               # (B,T,D)
    feat_tok = np.where(filled[..., None], tok, 0.0).transpose(0, 2, 1)
    return xyz_tok.astype(np.float32), feat_tok.astype(np.float32)


# revision 30
# speedup vs baseline: 1.1631x; 1.0341x over previous
"""AdaptiveDensityTokenizer on 8 TRN2 NeuronCores.

Strategy: the memory-bound importance MLP (reads all of `features`,
64 MB) is K-sharded across the 8 cores (4096 points/core); each core
computes softplus(relu(feat^T @ W1 + b1) @ W2 + b2) for its points.
The tiny data-dependent sequential logic (spatial bucketing, per-region
farthest-point sampling of ~256 total picks, concat/scatter) runs on
host, exactly replicating the reference semantics. The final token
projection (256 tokens x 256 feat @ Wa) is also host-side (0.03% of
the FLOPs).
"""

import os
import sys

import numpy as np

for _p in ("/opt/trn_rl_repo", "/root/.axon_site/_ro/trn_rl_repo"):
    if os.path.isdir(_p) and _p not in sys.path:
        sys.path.append(_p)

NCORES = 8
B, K, C, D = 2, 32768, 256, 256
T_TOK = 256
RPD = 3
R3 = 27
KC = K // NCORES          # points per core
CH = C // 128             # contraction chunks of 128
H = C // 2                # hidden width = 128
PT = 512                  # points per matmul tile (one PSUM bank of f32)

_NC = None                # cached compiled Bass program
LAST = None               # last BassKernelResults (for profiling)


def _build_mlp():
    """One SPMD program: per-core importance MLP over a K-shard.

    Layout/perf design (measured on neuron-profile traces):
    - features arrive host-packed as (128, 4*KC) bf16 so every DMA
      descriptor is a 4 KB contiguous row; staging is split into 8
      chunk DMAs alternating between the SP and ACT HWDGE rings (two
      descriptor generators) to feed all 16 SDMA engines.
    - a packed single-DMA weight load precedes the chunk triggers.
    - dependency-free scratch matmuls pre-heat the PE during the
      staging window so the HAM clock gate (1.2 -> 2.4 GHz after ~4us
      of sustained activity) lifts before real matmuls start.
    - mm1 accumulates C=256 in two 128-chunks into (128,1024) PSUM
      tiles; bias+relu drains alternate between ACT (fused activation)
      and DVE (fused tensor_scalar add+max); hs is written bf16.
    - the 16 (1,512) mm2 results land on PSUM partitions {0,32,64} of
      grouped tiles (PE writes must base at 0/32/64), drained eagerly
      by wide DVE copies + partition-strided DMAs; groups are sized
      {3,3,1,1} so the final drain chain after the last matmul is
      minimal.
    """
    from concourse import bacc, mybir, tile

    f32 = mybir.dt.float32
    bf16 = mybir.dt.bfloat16
    nc = bacc.Bacc(None, target_bir_lowering=False, debug=False)

    feat = nc.declare_dram_parameter("features", [128, 4 * KC], bf16, False)
    w1 = nc.declare_dram_parameter("W1", [128, C + 1], bf16, False)
    b1 = nc.declare_dram_parameter("b1", [H, 1], f32, False)
    out = nc.declare_dram_parameter("z", [B, KC], f32, True)

    # packed column ranges (must match _pack_features):
    #   [b0c0a | b0c1a | b0c0b | b0c1b | b1c0a | b1c1a | b1c0b | b1c1b]
    # 2048-wide chunks (4 KB descriptor rows) measured faster than any
    # 4096-wide variant: descriptor generation scales with size, and
    # finer granularity keeps the PE fed earlier.
    CHUNKS = [
        ("A0", 0, 2048), ("A1", 2048, 2048),
        ("B0", 4096, 2048), ("B1", 6144, 2048),
        ("C0", 8192, 2048), ("C1", 10240, 2048),
        ("D0", 12288, 2048), ("D1", 14336, 2048),
    ]

    with tile.TileContext(nc) as tc:
        with (
            tc.tile_pool(name="wpool", bufs=1) as wpool,
            tc.tile_pool(name="fpool", bufs=1) as fpool,
            tc.tile_pool(name="hpool", bufs=3) as hpool,
            tc.tile_pool(name="zpool", bufs=2) as zpool,
            tc.tile_pool(name="ph", bufs=2, space="PSUM") as phpool,
            tc.tile_pool(name="pz", bufs=1, space="PSUM") as pzpool,
        ):
            # Weights first (single packed DMA on the sync ring; they gate
            # the first real matmuls), then the staging chunk triggers so
            # both HWDGE rings stream features immediately after.
            wall = wpool.tile([128, C + 1], bf16, tag="wall")
            nc.sync.dma_start(out=wall[:], in_=w1[:, :])
            b1t = wpool.tile([H, 1], f32, tag="b1")
            nc.scalar.dma_start(out=b1t[:], in_=b1[:, :])
            w1t = [wall[:, c * 128:(c + 1) * 128] for c in range(CH)]
            w2t = wall[:, C:C + 1]

            ftiles = {}
            for i, (name, off, width) in enumerate(CHUNKS):
                t = fpool.tile([128, width], bf16, tag=f"f_{name}")
                eng = nc.sync if i % 2 == 0 else nc.scalar
                eng.dma_start(out=t[:], in_=feat[:, off:off + width])
                ftiles[name] = t

            def rhs(b, c, j):
                # feature columns [j, j+512) of chunk c for batch b
                names = (("A0", "A1", "B0", "B1"), ("C0", "C1", "D0", "D1"))
                t = ftiles[names[b][(j // 2048) * 2 + c]]
                return t[:, j % 2048:j % 2048 + 512]

            # Pre-heat the PE during the staging window: the HAM clock
            # gate needs ~4us of sustained activity to lift 1.2->2.4 GHz,
            # and the first real matmul can't start until weights+chunks
            # land (~10us). Scratch matmuls on uninitialized tiles have no
            # dependencies and keep the array busy from right after the
            # start barrier, so real work runs at the warm clock.
            wscr = wpool.tile([128, 128], bf16, tag="wscr")
            mscr = wpool.tile([128, PT], bf16, tag="mscr")
            nc.vector.memset(wscr[:], 0.0)
            nc.vector.memset(mscr[:], 0.0)
            pwarm = pzpool.tile([65, PT], f32, tag="pz0", name="pwarm")
            for _ in range(14):
                nc.tensor.matmul(pwarm[0:1, :], wscr[:, 0:1], mscr[:],
                                 start=True, stop=True)

            relu = mybir.ActivationFunctionType.Relu
            add = mybir.AluOpType.add
            amax = mybir.AluOpType.max
            for b in range(B):
                # mm2 outputs must land on PSUM base partition 0/32/64:
                # pack 3 per PSUM tile, drain with partition-strided copies.
                pzs = []
                for t in range(4):
                    pzt = pzpool.tile([65, PT], f32, tag=f"pz{t}", name=f"pz{t}_{b}")
                    pzs.append(pzt)
                for i in range(4):                      # 1024-pt double tiles
                    ph = phpool.tile([128, 2 * PT], f32)
                    hs = hpool.tile([128, 2 * PT], bf16)
                    for half in range(2):
                        sl = slice(half * PT, (half + 1) * PT)
                        j = i * 1024 + half * PT
                        for c in range(CH):
                            nc.tensor.matmul(
                                ph[:, sl], w1t[c], rhs(b, c, j),
                                start=(c == 0), stop=(c == CH - 1),
                            )
                        # b0: split relus ACT/DVE. b1: all ACT — by then the
                        # DVE is the busy engine (it still owes the pz/zsb
                        # drains) and PE stalls waiting on hs otherwise.
                        if half == 0 or b == 1:
                            nc.scalar.activation(hs[:, sl], ph[:, sl], relu,
                                                 bias=b1t[:])
                        else:
                            nc.vector.tensor_scalar(
                                out=hs[:, sl], in0=ph[:, sl],
                                scalar1=b1t[:], scalar2=0.0,
                                op0=add, op1=amax,
                            )
                        r = 2 * i + half
                        grp = (0, 0, 0, 1, 1, 1, 2, 3)[r]
                        base = (0, 3, 6, 7)[grp]
                        row = (r - base) * 32
                        nc.tensor.matmul(
                            pzs[grp][row:row + 1, :], w2t, hs[:, sl],
                            start=True, stop=True,
                        )
                        if r in (2, 5, 6, 7):   # group full: drain eagerly
                            grp = (0, 0, 0, 1, 1, 1, 2, 3)[r]
                            nrow = (3, 3, 1, 1)[grp]
                            zview = out[b].rearrange("(p f) -> p f", p=8)
                            zt = zpool.tile([65, PT], f32, tag=f"zsb{grp}",
                                            name=f"zsb{grp}_{b}")
                            nc.vector.tensor_copy(
                                zt[0:(nrow - 1) * 32 + 1, :],
                                pzs[grp][0:(nrow - 1) * 32 + 1, :])
                            nc.sync.dma_start(
                                out=zview[base:base + nrow, :],
                                in_=zt[0:(nrow - 1) * 32 + 1:32, :],
                            )

    nc.compile()
    return nc


def _pack_features(fbf, core):
    """(B,C,K) bf16 -> per-core (128, 4*KC) packed layout; see CHUNKS."""
    sh = fbf[:, :, core * KC:(core + 1) * KC].reshape(B, CH, 128, KC)
    p = np.empty((128, 4 * KC), fbf.dtype)
    p[:, 0:2048] = sh[0, 0, :, 0:2048]
    p[:, 2048:4096] = sh[0, 1, :, 0:2048]
    p[:, 4096:6144] = sh[0, 0, :, 2048:4096]
    p[:, 6144:8192] = sh[0, 1, :, 2048:4096]
    p[:, 8192:10240] = sh[1, 0, :, 0:2048]
    p[:, 10240:12288] = sh[1, 1, :, 0:2048]
    p[:, 12288:14336] = sh[1, 0, :, 2048:4096]
    p[:, 14336:16384] = sh[1, 1, :, 2048:4096]
    return p


def _get_nc():
    global _NC
    if _NC is None:
        _NC = _build_mlp()
    return _NC


def _ensure_profile_hook():
    """Shim antenv.axon_hooks (absent in this image) so the trace=True
    path of run_bass_kernel_spmd can capture NTFF profiles, and stub the
    S3 artifact upload. Only used when BASS_PROFILE=1."""
    import types

    try:
        import antenv.axon_hooks  # noqa: F401
    except ImportError:
        try:
            import antenv
            from trn_agent_boot.trn_boot import _ntff_profile_via_ctypes

            hook = _ntff_profile_via_ctypes("/opt/axon/libaxon_pjrt.so")
            mod = types.ModuleType("antenv.axon_hooks")
            mod.get_axon_ntff_profile_hook = lambda: hook
            mod.set_axon_ntff_profile_hook = lambda h: None
            sys.modules["antenv.axon_hooks"] = mod
            antenv.axon_hooks = mod
        except Exception:
            return False
    import concourse.bass_utils as bu

    bu.upload_artifacts = lambda d: "file://" + d
    return True


def _device_importance(features, W1, b1, W2, b2):
    from concourse.bass_utils import run_bass_kernel_spmd

    global LAST
    nc = _get_nc()
    profile = bool(int(os.environ.get("BASS_PROFILE", "0")))
    if profile:
        profile = _ensure_profile_hook()
    import ml_dtypes

    bf = ml_dtypes.bfloat16
    w1p = np.empty((128, C + 1), bf)
    w1p[:, 0:H] = np.ascontiguousarray(W1[0:128]).astype(bf)
    w1p[:, H:C] = np.ascontiguousarray(W1[128:256]).astype(bf)
    w1p[:, C] = np.ascontiguousarray(W2).astype(bf).reshape(H)
    b1c = np.ascontiguousarray(b1, np.float32).reshape(H, 1)
    fbf = np.asarray(features, np.float32).astype(bf)
    in_maps = []
    for core in range(NCORES):
        sl = slice(core * KC, (core + 1) * KC)
        in_maps.append({
            "features": _pack_features(fbf, core),
            "W1": w1p, "b1": b1c,
        })
    res = run_bass_kernel_spmd(
        nc, in_maps, core_ids=list(range(NCORES)), trace=profile,
    )
    LAST = res
    z = np.concatenate([res.results[c]["z"] for c in range(NCORES)], axis=1)
    z = z + np.float32(np.asarray(b2).reshape(()))
    # softplus on host with the exact jax.nn.softplus formula in f32
    # (the ACT engine's Softplus LUT is not precise enough for the
    # data-dependent n_r rounding margins).
    return np.maximum(z, 0) + np.log1p(np.exp(-np.abs(z)))


def _fps_region(pts, n_steps):
    """Farthest-point sampling over one compacted region, mirroring the
    reference: start at subset index 0, squared L2, first-max argmax."""
    n = pts.shape[0]
    picks = np.empty(n_steps, np.int64)
    mind = np.full(n, np.float32(1e10), np.float32)
    p = 0
    x, y, z = pts[:, 0], pts[:, 1], pts[:, 2]
    for s in range(n_steps):
        picks[s] = p
        dx = x - x[p]
        dy = y - y[p]
        dz = z - z[p]
        d = (dx * dx + dy * dy) + dz * dz
        np.minimum(mind, d, out=mind)
        p = int(np.argmax(mind))
    return picks


def kernel(xyz, features, W1, b1, W2, b2, Wa, ba):
    xyz = np.asarray(xyz, np.float32)
    features = np.asarray(features, np.float32)
    Wa = np.asarray(Wa, np.float32)
    ba = np.asarray(ba, np.float32)

    imp = _device_importance(features, W1, b1, W2, b2)        # (B, K)

    # ---- spatial bucketing (exact reference semantics, f32 ops) ----
    mn = xyz.min(axis=1, keepdims=True)
    mx = xyz.max(axis=1, keepdims=True)
    xn = (xyz - mn) / (mx - mn + np.float32(1e-6))
    ridx = np.clip(xn * np.float32(RPD), 0, RPD - 1).astype(np.int32)
    rid = ridx[..., 0] * RPD * RPD + ridx[..., 1] * RPD + ridx[..., 2]
    valid = np.abs(xyz).sum(-1) > 0                           # (B, K)

    onehot = (rid[..., None] == np.arange(R3)) & valid[..., None]
    counts = onehot.sum(axis=1).astype(np.int32)              # (B, R3)
    reg_imp = np.einsum(
        "bk,bkr->br",
        (imp * valid).astype(np.float32),
        onehot.astype(np.float32),
    )
    share = reg_imp / (reg_imp.sum(-1, keepdims=True) + np.float32(1e-8))
    n_r = np.round(share * np.float32(T_TOK)).astype(np.int32)
    c_r = np.where(n_r == 0, 0, np.minimum(n_r, counts))      # (B, R3)

    # ---- per-region selection: FPS picks or ascending slab ----
    out_idx = np.zeros((B, T_TOK), np.int32)
    filled = np.zeros((B, T_TOK), bool)
    for b in range(B):
        start = 0
        for r in range(R3):
            c = int(c_r[b, r])
            if c == 0:
                continue
            members = np.nonzero(onehot[b, :, r])[0]          # ascending
            if counts[b, r] <= n_r[b, r]:
                sel = members[:c]
            else:
                pts = xyz[b][members]
                sel = members[_fps_region(pts, c)]
            take = min(c, T_TOK - start)
            if take > 0:
                out_idx[b, start:start + take] = sel[:take]
                filled[b, start:start + take] = True
            start += c
            if start >= T_TOK:
                break

    # ---- gather + output heads ----
    xyz_tok = np.where(
        filled[..., None], np.take_along_axis(xyz, out_idx[..., None], axis=1), 0.0
    ).astype(np.float32)

    gath = np.stack([features[b][:, out_idx[b]].T for b in range(B)])  # (B,T,C)
    tok = gath @ Wa + ba                                               # (B,T,D)
    feat_tok = np.where(filled[..., None], tok, 0.0).transpose(0, 2, 1)
    return xyz_tok.astype(np.float32), feat_tok.astype(np.float32)
